# revision 39
# baseline (speedup 1.0000x reference)
import sys

for _p in ("/opt/trn_rl_repo",):
    if _p not in sys.path:
        sys.path.insert(0, _p)

import numpy as np
import ml_dtypes

B, S, D = 2, 2048, 1024
H, HD = 16, 64
FF = 4 * D
T = B * S
NCORES = 8
TC = T // NCORES
P = 128
KT = D // P
KA = 9
DAUG = D + P
EPS = 1e-5
NQC = 4
NKT = S // P
BF16 = ml_dtypes.bfloat16

_CACHE = {}


def _build_program(has_c1, has_bproj, has_c2, has_b1, has_b2):
    import concourse.mybir as mybir
    import concourse.tile as tile
    from concourse import bacc
    from concourse.masks import make_identity
    from contextlib import ExitStack

    f32 = mybir.dt.float32
    bf16 = mybir.dt.bfloat16
    AF = mybir.ActivationFunctionType
    ALU = mybir.AluOpType

    nc = bacc.Bacc(None, target_bir_lowering=False)

    x_aug_d = nc.declare_dram_parameter("x_aug", [DAUG, T], bf16, isOutput=False)
    x_c_d = nc.declare_dram_parameter("x_c", [D, TC], f32, isOutput=False)
    wqkv_d = nc.declare_dram_parameter("wqkv_aug", [DAUG, 3 * P], bf16, isOutput=False)
    wproj_d = nc.declare_dram_parameter("wproj_c", [P, D], bf16, isOutput=False)
    w1_d = nc.declare_dram_parameter("w1_aug", [DAUG, FF], bf16, isOutput=False)
    w2t_d = nc.declare_dram_parameter("w2t", [FF, D], bf16, isOutput=False)
    aux_d = nc.declare_dram_parameter("aux", [P, 64], f32, isOutput=False)
    out_d = nc.declare_dram_parameter("out_c", [D, TC], f32, isOutput=True)

    core_ids = list(range(NCORES))

    with tile.TileContext(nc) as tc, ExitStack() as ctx:
        const = ctx.enter_context(tc.tile_pool(name="const", bufs=1))
        dram = ctx.enter_context(tc.tile_pool(name="dram", bufs=1, space="DRAM"))

        ident = const.tile([P, P], bf16)
        make_identity(nc, ident)
        ones128 = const.tile([P, P], bf16)
        nc.any.memset(ones128, 1.0)
        eps_col = const.tile([P, 1], f32)
        nc.any.memset(eps_col, EPS)

        wqkv_sb = const.tile([P, KA, 3 * P], bf16)
        nc.sync.dma_start(wqkv_sb, wqkv_d.rearrange("(k p) e -> p k e", p=P))
        wproj_sb = const.tile([P, D], bf16)
        nc.sync.dma_start(wproj_sb, wproj_d[:])
        aux_sb = const.tile([P, 64], f32)
        nc.sync.dma_start(aux_sb, aux_d[:])

        x1grp = ctx.enter_context(tc.tile_pool(name="x1grp", bufs=1))
        x1 = x1grp.tile([P, KT, TC], f32)
        x1aug = x1grp.tile([P, KA, TC], bf16)
        r2b = x1grp.tile([P, TC], f32)
        h2T = ctx.enter_context(tc.tile_pool(name="h2", bufs=1)).tile(
            [P, FF // P, TC], bf16
        )
        work = ctx.enter_context(tc.tile_pool(name="work", bufs=2))

        psA = ctx.enter_context(tc.tile_pool(name="psA", bufs=2, space="PSUM"))
        psB = ctx.enter_context(tc.tile_pool(name="psB", bufs=3, space="PSUM"))

        partial_all = dram.tile([NCORES * D, TC], bf16, tag="pall", name="pall")
        partial_b = [partial_all[0 : 4 * D, :], partial_all[4 * D :, :]]
        x1proj_d = dram.tile([D, TC], bf16, tag="x1proj", name="x1proj")

        x_aug_r = x_aug_d.rearrange("(k p) t -> p k t", p=P)
        w1_noaug_r = w1_d.rearrange("(k p) f -> p k f", p=P)

        with tc.tile_pool(name="qkvTp", bufs=1) as qkvT_pool, \
             tc.tile_pool(name="attnTp", bufs=1) as attnT_pool, \
             tc.tile_pool(name="attg", bufs=1) as attg, \
             tc.tile_pool(name="etp", bufs=8) as etp, \
             tc.tile_pool(name="poutp", bufs=6) as poutp:
            qkvT = qkvT_pool.tile([P, 3, T], bf16)
            attnT = attnT_pool.tile([P, T], bf16)

            with tc.tile_pool(name="xaug", bufs=2) as xaug_pool, \
                 nc.named_scope("ln1_qkv"):
                for tch in range(T // 512):
                    tsl = slice(tch * 512, (tch + 1) * 512)
                    xa = xaug_pool.tile([P, KA, 512], bf16, tag="xa")
                    nc.sync.dma_start(xa, x_aug_r[:, :, tsl])
                    pmu = psA.tile([P, 512], f32, tag="a", name="pmu")
                    psq = psA.tile([P, 512], f32, tag="a", name="psq")
                    for kt in range(KT):
                        xsq = work.tile([P, 512], bf16, tag="xsq")
                        nc.vector.tensor_tensor(
                            xsq, xa[:, kt, :], xa[:, kt, :], ALU.mult
                        )
                        nc.tensor.matmul(
                            pmu, ones128, xa[:, kt, :],
                            start=(kt == 0), stop=(kt == KT - 1),
                        )
                        nc.tensor.matmul(
                            psq, ones128, xsq,
                            start=(kt == 0), stop=(kt == KT - 1),
                        )
                    m1 = work.tile([P, 512], f32, tag="m1")
                    nc.vector.tensor_scalar_mul(m1, pmu, 1.0 / D)
                    nc.vector.tensor_scalar_mul(xa[0:1, KT, :], m1[0:1, :], -1.0)
                    v1 = work.tile([P, 512], f32, tag="v1")
                    nc.vector.tensor_scalar_mul(v1, psq, 1.0 / D)
                    m2 = work.tile([P, 512], f32, tag="m2")
                    nc.vector.tensor_tensor(m2, m1, m1, ALU.mult)
                    nc.vector.tensor_tensor(v1, v1, m2, ALU.subtract)
                    sd = work.tile([P, 512], f32, tag="sd")
                    nc.scalar.activation(sd, v1, AF.Sqrt, bias=eps_col)
                    r1b = xaug_pool.tile([P, 512], f32, tag="r1b")
                    nc.vector.reciprocal_approx_fast(r1b, sd)

                    for m in range(3):
                        msl = slice(m * P, (m + 1) * P)
                        ps = psA.tile([P, 512], f32, tag="a", name="qkvps")
                        for kt in range(KA):
                            nc.tensor.matmul(
                                ps, wqkv_sb[:, kt, msl], xa[:, kt, :],
                                start=(kt == 0), stop=(kt == KA - 1),
                            )
                        nc.vector.tensor_tensor(qkvT[:, m, tsl], ps, r1b, ALU.mult)
                        if has_c1:
                            nc.vector.tensor_scalar(
                                qkvT[:, m, tsl], qkvT[:, m, tsl],
                                aux_sb[:, 48 + m : 49 + m], None, ALU.add,
                            )

            vext = attg.tile([P, B * NKT, 130], bf16)
            with nc.named_scope("vext"):
                nc.any.memset(vext[:, :, 64:65], 1.0)
                nc.any.memset(vext[:, :, 129:130], 1.0)
                for b in range(B):
                    for kt in range(NKT):
                        pt = psA.tile([P, 512], bf16, tag="a", name="pt")[:, 0:P]
                        nc.tensor.transpose(
                            pt, qkvT[:, 2, b * S + kt * P : b * S + (kt + 1) * P],
                            ident,
                        )
                        nc.vector.tensor_copy(vext[:, b * NKT + kt, 0:64], pt[:, 0:64])
                        nc.vector.tensor_copy(
                            vext[:, b * NKT + kt, 65:129], pt[:, 64:128]
                        )

            with nc.named_scope("attn"):
                from collections import deque

                avqs = {}
                pend = deque()
                epi_q = deque()

                def _epilogue(b, h, qc2):
                    avq = avqs.pop((b, h, qc2))
                    q0 = b * S + qc2 * 1024
                    rs_sb = attg.tile([1, 1024], f32, tag="rs_sb", name="rs_sb")
                    nc.vector.tensor_copy(rs_sb, avq[64:65, :])
                    rc = attg.tile([1, 1024], f32, tag="rc", name="rc")
                    nc.vector.reciprocal_approx_fast(rc, rs_sb)
                    rcb = attg.tile([1, 1024], bf16, tag="rcb", name="rcb")
                    nc.vector.tensor_copy(rcb, rc)
                    rbp = psB.tile([P, 1024], f32, tag="b", name="rbp")[0:64, :]
                    for half in range(2):
                        nc.tensor.matmul(
                            rbp[:, half * 512 : (half + 1) * 512],
                            ones128[0:1, 0:64],
                            rcb[0:1, half * 512 : (half + 1) * 512],
                            start=True, stop=True,
                        )
                    rbs = attg.tile([64, 1024], bf16, tag="rb", name="rbs")
                    nc.vector.tensor_copy(rbs, rbp)
                    nc.vector.tensor_tensor(
                        attnT[h * HD : (h + 1) * HD, q0 : q0 + 1024],
                        avq[0:64, :], rbs, ALU.mult,
                    )

                def _emit_av(e):
                    b, h, qc2, kt, et = e
                    avq = avqs[(b, h, qc2)]
                    vcol = slice(h * 65, h * 65 + 65)
                    for half in range(2):
                        nc.tensor.matmul(
                            avq[:, half * 512 : (half + 1) * 512],
                            vext[:, b * NKT + kt, vcol],
                            et[:, half * 512 : (half + 1) * 512],
                            start=(kt == 0), stop=(kt == NKT - 1),
                        )
                    if kt == NKT - 1:
                        epi_q.append((b, h, qc2))

                proj_q = deque()

                def _proj_ops(b):
                    ops = []
                    for m in range(D // P):
                        for tch in range(b * 4, b * 4 + 4):
                            ops.append((b, m, tch))
                    return ops

                def _emit_rs(b):
                    if b == B - 1:
                        with nc.named_scope("reducescatter"):
                            nc.gpsimd.collective_compute(
                                "ReduceScatter",
                                mybir.AluOpType.add,
                                replica_groups=[core_ids],
                                ins=[partial_all[:]],
                                outs=[x1proj_d[:]],
                            )

                def _emit_proj():
                    b, m, tch = proj_q.popleft()
                    tsl = slice(tch * TC, (tch + 1) * TC)
                    ps = psA.tile([P, 512], f32, tag="a", name="projps")
                    nc.tensor.matmul(
                        ps, wproj_sb[:, m * P : (m + 1) * P],
                        attnT[:, tsl], start=True, stop=True,
                    )
                    pb = poutp.tile([P, TC], bf16, tag="pout", name="pb")
                    nc.vector.tensor_copy(pb, ps)
                    nc.sync.dma_start(
                        partial_b[b][
                            (tch - b * 4) * D + m * P
                            : (tch - b * 4) * D + (m + 1) * P, :
                        ],
                        pb,
                    )
                    if not proj_q:
                        _emit_rs(b)

                bsteps = 0
                for b in range(B):
                    bsteps = 0
                    for h in range(2):
                        hsl = slice(h * HD, (h + 1) * HD)
                        for qc2 in range(2):
                            q0 = b * S + qc2 * 1024
                            avqs[(b, h, qc2)] = psB.tile(
                                [65, 1024], f32, tag="b", name=f"avq{b}{h}{qc2}"
                            )
                            for kt in range(NKT):
                                ksl = slice(b * S + kt * P, b * S + (kt + 1) * P)
                                lg = psB.tile([P, 1024], f32, tag="b", name="lg")
                                for half in range(2):
                                    nc.tensor.matmul(
                                        lg[:, half * 512 : (half + 1) * 512],
                                        qkvT[hsl, 1, ksl],
                                        qkvT[hsl, 0, q0 + half * 512 : q0 + (half + 1) * 512],
                                        start=True, stop=True,
                                    )
                                et = etp.tile([P, 1024], bf16, tag="et")
                                nc.scalar.activation(
                                    et, lg, AF.Exp, scale=1.0 / np.sqrt(HD)
                                )
                                pend.append((b, h, qc2, kt, et))
                                while len(pend) > 4:
                                    _emit_av(pend.popleft())
                                if len(epi_q) > 1 or (epi_q and (kt > 2 or bsteps > 4)):
                                    _epilogue(*epi_q.popleft())
                                if proj_q and bsteps > 6:
                                    _emit_proj()
                                bsteps += 1
                    proj_q.extend(_proj_ops(b))
                    if b == B - 1:
                        while pend:
                            _emit_av(pend.popleft())
                        while epi_q:
                            _epilogue(*epi_q.popleft())
                        with nc.named_scope("proj"):
                            while proj_q:
                                _emit_proj()

        w_stack = ExitStack()
        w_stack = ExitStack()
        w_stack = ExitStack()
        w_pool = w_stack.enter_context(tc.tile_pool(name="wpool", bufs=1))
        FQ = FF // 4
        w1q = []
        for q in range(4):
            w1qt = w_pool.tile([P, KA, FQ], bf16, tag=f"w{q}", name=f"w1q{q}")
            nc.sync.dma_start(w1qt, w1_noaug_r[:, :, q * FQ : (q + 1) * FQ])
            w1q.append(w1qt)

        with tc.tile_pool(name="resid", bufs=1) as resid, \
             tc.tile_pool(name="x1pp", bufs=2) as x1pp, nc.named_scope("x1_ln2"):
            xc = resid.tile([P, KT, TC], f32, tag="xc")
            nc.sync.dma_start(xc, x_c_d.rearrange("(k p) t -> p k t", p=P))
            nc.any.memset(x1aug[:, KT, :], 0.0)
            pmu = psA.tile([P, 512], f32, tag="a", name="pmu2")
            psq = psA.tile([P, 512], f32, tag="a", name="psq2")
            x1p_r = x1proj_d.rearrange("(k p) t -> p k t", p=P)
            for g in range(4):
                x1p = x1pp.tile([P, 2, TC], bf16, tag="x1p")
                nc.sync.dma_start(x1p, x1p_r[:, 2 * g : 2 * g + 2, :])
                for sub in range(2):
                    kt = 2 * g + sub
                    nc.vector.tensor_tensor(
                        x1[:, kt, :], xc[:, kt, :], x1p[:, sub, :], ALU.add
                    )
                    if has_bproj:
                        nc.vector.tensor_scalar(
                            x1[:, kt, :], x1[:, kt, :],
                            aux_sb[:, kt : kt + 1], None, ALU.add,
                        )
                    nc.vector.tensor_copy(x1aug[:, kt, :], x1[:, kt, :])
                    xsq = work.tile([P, TC], bf16, tag="xsq")
                    nc.vector.tensor_tensor(
                        xsq, x1aug[:, kt, :], x1aug[:, kt, :], ALU.mult
                    )
                    nc.tensor.matmul(
                        pmu, ones128, x1aug[:, kt, :],
                        start=(kt == 0), stop=(kt == KT - 1),
                    )
                    nc.tensor.matmul(
                        psq, ones128, xsq, start=(kt == 0), stop=(kt == KT - 1)
                    )
            m1 = work.tile([P, TC], f32, tag="m1")
            nc.vector.tensor_scalar_mul(m1, pmu, 1.0 / D)
            nc.vector.tensor_scalar_mul(x1aug[0:1, KT, :], m1[0:1, :], -1.0)
            v1 = work.tile([P, TC], f32, tag="v1")
            nc.vector.tensor_scalar_mul(v1, psq, 1.0 / D)
            m2 = work.tile([P, TC], f32, tag="m2")
            nc.vector.tensor_tensor(m2, m1, m1, ALU.mult)
            nc.vector.tensor_tensor(v1, v1, m2, ALU.subtract)
            sd = work.tile([P, TC], f32, tag="sd")
            nc.scalar.activation(sd, v1, AF.Sqrt, bias=eps_col)
            nc.vector.reciprocal_approx_fast(r2b, sd)

        NF = FF // P
        NQ = NF // 4
        w2r = w2t_d.rearrange("(k p) d -> p k d", p=P)
        w2q = [None] * 4

        def _w2s(kt, m):
            return w2q[kt // NQ][:, kt % NQ, m * P : (m + 1) * P]

        accs3 = [
            psB.tile([P, 1024], f32, tag="b", name=f"m2p{g}") for g in range(3)
        ]
        accs = [accs3[m // 2][:, (m % 2) * TC : (m % 2 + 1) * TC] for m in range(6)]
        with nc.named_scope("mlp"):
            assert not has_c2, "nonzero ln2_b not supported"
            for j in range(NF):
                if j >= NQ and j % NQ == 0:
                    q = j // NQ - 1
                    w2q[q] = w_pool.tile(
                        [P, NQ, D], bf16, tag=f"w{q}", name=f"w2q{q}"
                    )
                    nc.sync.dma_start(w2q[q], w2r[:, q * NQ : (q + 1) * NQ, :])
                w1h = w1q[j // NQ]
                msl = slice((j % NQ) * P, (j % NQ + 1) * P)
                ps = psA.tile([P, 512], f32, tag="a", name="m1ps")
                for kt in range(KA):
                    nc.tensor.matmul(
                        ps, w1h[:, kt, msl], x1aug[:, kt, :],
                        start=(kt == 0), stop=(kt == KA - 1),
                    )
                z1 = work.tile([P, TC], f32, tag="z1")
                nc.vector.tensor_tensor(z1, ps, r2b, ALU.mult)
                bias_arg = aux_sb[:, 8 + j : 9 + j] if has_b1 else 0.0
                nc.scalar.activation(h2T[:, j, :], z1, AF.Relu, bias=bias_arg)
                if j >= 2 * NQ:
                    kt2 = j - 2 * NQ
                    for m in range(6):
                        nc.tensor.matmul(
                            accs[m], _w2s(kt2, m), h2T[:, kt2, :],
                            start=(kt2 == 0), stop=False,
                        )
            w2q[3] = w_pool.tile([P, NQ, D], bf16, tag="w3", name="w2q3")
            nc.sync.dma_start(w2q[3], w2r[:, 3 * NQ :, :])
            for kt2 in range(NF - 2 * NQ, NF):
                for m in range(6):
                    nc.tensor.matmul(
                        accs[m], _w2s(kt2, m), h2T[:, kt2, :],
                        start=False, stop=(kt2 == NF - 1),
                    )
            for m in range(6):
                acc = accs[m]
                ob = work.tile([P, TC], f32, tag="ob")
                nc.vector.tensor_tensor(ob, acc, x1[:, m, :], ALU.add)
                if has_b2:
                    nc.vector.tensor_scalar(
                        ob, ob, aux_sb[:, 40 + m : 41 + m], None, ALU.add
                    )
                nc.sync.dma_start(out_d[m * P : (m + 1) * P, :], ob)
            tails = [
                psA.tile([P, 512], f32, tag="a", name=f"m2t{m}") for m in range(2)
            ]
            for kt in range(NF):
                for m in range(2):
                    nc.tensor.matmul(
                        tails[m], _w2s(kt, 6 + m), h2T[:, kt, :],
                        start=(kt == 0), stop=(kt == NF - 1),
                    )
            for m in range(2):
                ob = work.tile([P, TC], f32, tag="ob")
                nc.vector.tensor_tensor(ob, tails[m], x1[:, 6 + m, :], ALU.add)
                if has_b2:
                    nc.vector.tensor_scalar(
                        ob, ob, aux_sb[:, 46 + m : 47 + m], None, ALU.add
                    )
                nc.sync.dma_start(out_d[(6 + m) * P : (7 + m) * P, :], ob)
        w_stack.close()

    nc.compile()
    return nc


def _prep_inputs(inputs):
    x = np.asarray(inputs["x"], np.float32)
    w_qkv = np.asarray(inputs["w_qkv"], np.float32)
    w_proj = np.asarray(inputs["w_proj"], np.float32)
    b_proj = np.asarray(inputs["b_proj"], np.float32)
    w1 = np.asarray(inputs["w1"], np.float32)
    b1 = np.asarray(inputs["b1"], np.float32)
    w2 = np.asarray(inputs["w2"], np.float32)
    b2 = np.asarray(inputs["b2"], np.float32)
    ln1_g = np.asarray(inputs["ln1_g"], np.float32)
    ln1_b = np.asarray(inputs["ln1_b"], np.float32)
    ln2_g = np.asarray(inputs["ln2_g"], np.float32)
    ln2_b = np.asarray(inputs["ln2_b"], np.float32)

    has_c1 = bool(np.any(ln1_b != 0))
    has_bproj = bool(np.any(b_proj != 0))
    has_c2 = bool(np.any(ln2_b != 0))
    has_b1 = bool(np.any(b1 != 0))
    has_b2 = bool(np.any(b2 != 0))
    flags = (has_c1, has_bproj, has_c2, has_b1, has_b2)

    xT = np.ascontiguousarray(x.reshape(T, D).T)
    x_aug = np.zeros((DAUG, T), BF16)
    x_aug[:D] = xT.astype(BF16)

    wg = w_qkv * ln1_g[None, :]
    Se = wg.sum(axis=1)
    Ce = w_qkv @ ln1_b
    w1g = w1 * ln2_g[None, :]
    S1 = w1g.sum(axis=1)
    C2 = w1 @ ln2_b
    if np.any(C2 != 0):
        raise NotImplementedError("nonzero ln2_b not supported")

    w1_aug = np.zeros((DAUG, FF), BF16)
    w1_aug[:D] = w1g.T.astype(BF16)
    w1_aug[D] = S1.astype(BF16)
    w2t = np.ascontiguousarray(w2.T).astype(BF16)

    in_maps = []
    for c in range(NCORES):
        rsl = slice(c * P, (c + 1) * P)
        wstack = np.concatenate([wg[rsl], wg[D:][rsl], wg[2 * D :][rsl]], axis=0)
        sstack = np.concatenate([Se[rsl], Se[D:][rsl], Se[2 * D :][rsl]])
        cstack = np.concatenate([Ce[rsl], Ce[D:][rsl], Ce[2 * D :][rsl]])
        wqkv_aug = np.zeros((DAUG, 3 * P), BF16)
        wqkv_aug[:D] = wstack.T.astype(BF16)
        wqkv_aug[D] = sstack.astype(BF16)

        wproj_c = np.ascontiguousarray(w_proj[:, c * P : (c + 1) * P].T).astype(BF16)

        aux = np.zeros((P, 64), np.float32)
        aux[:, 0:8] = b_proj.reshape(KT, P).T
        aux[:, 8:40] = b1.reshape(FF // P, P).T
        aux[:, 40:48] = b2.reshape(KT, P).T
        aux[:, 48:51] = cstack.reshape(3, P).T
        aux[:, 51] = 1.0 if c < 4 else 0.0
        aux[:, 52] = 0.0 if c < 4 else 1.0

        in_maps.append(
            {
                "x_aug": x_aug,
                "x_c": np.ascontiguousarray(xT[:, c * TC : (c + 1) * TC]),
                "wqkv_aug": wqkv_aug,
                "wproj_c": wproj_c,
                "w1_aug": w1_aug,
                "w2t": w2t,
                "aux": aux,
            }
        )
    return flags, in_maps


def _run(inputs, trace=False, trace_kwargs=None):
    from concourse.bass_utils import run_bass_kernel_spmd

    flags, in_maps = _prep_inputs(inputs)
    if flags not in _CACHE:
        _CACHE[flags] = _build_program(*flags)
    nc = _CACHE[flags]
    res = run_bass_kernel_spmd(
        nc, in_maps, list(range(NCORES)), trace=trace,
        **(trace_kwargs or {}),
    )
    outT = np.empty((D, T), np.float32)
    for c in range(NCORES):
        outT[:, c * TC : (c + 1) * TC] = res.results[c]["out_c"]
    out = np.ascontiguousarray(outT.T).reshape(B, S, D)
    return out, res


def kernel(**inputs):
    out, _ = _run(inputs, trace=False)
    return out
